# revision 7
# baseline (speedup 1.0000x reference)
"""Causal self-attention (GQA + QK-RMSNorm + RoPE + q_gain) on 8 Trainium2 cores.

Sharding: 8 cores = 2 (batch) x 4 (KV head group).  Core c handles batch
c//4 and KV head g=c%4, i.e. Q heads 4g..4g+3.  Each core computes its
heads' attention and a partial output projection (its 512 columns of the
attention output against the matching 512 rows of Wproj^T); the host sums
the 4 partials per batch.

Single streamed loop over 512-token slices: QKV projection (+ QK-RMSNorm +
RoPE) for slice js, then causal attention for query slice js (keys 0..js),
then the output projection for those tokens.  The Tile scheduler overlaps
the phases across slices.

Datapath: fp32r for the QKV projections (x and W stay fp32), bf16 for
q/k/v, attention probabilities, attention output and Wproj (all matmuls
stay at 1 PE cycle per output column; bf16 doubles DVE throughput and
halves SBUF).  The causal mask is preloaded into PSUM (additive -1e30)
so masking costs nothing on the exp->PV critical chain.  RMSNorm gain and
1/sqrt(hd) are folded into exp(-0.5*ln(z)) on the scalar engine, which
also keeps every activation in one table set (no table reloads).

All shapes hardcoded for B=2, S=2048, D=2048, H=16, KVH=4, HD=128.
"""

import os

import numpy as np

B, S, D = 2, 2048, 2048
H, KVH = 16, 4
HD = 128  # head dim
G = H // KVH  # q heads per kv group = 4
NCORES = 8
ROPE_BASE = 10000.0
EPS = 1e-6

P = 128          # partitions
SL = 512         # token slice
NSL = S // SL    # 4
DK = D // P      # 16 contraction subtiles
HK = DK // 2     # x arrives in half-slices of 8 subtiles

# psum pool buffer knobs (env-overridable for perf sweeps; defaults are the
# shipped configuration and must sum to <= 8 banks with rs+o+vtr)
QKV_BUFS = int(os.environ.get("K_QKV", "2"))
NRM_BUFS = int(os.environ.get("K_NRM", "2"))
SC_BUFS = int(os.environ.get("K_SC", "2"))
MASK_ENG = os.environ.get("K_MASKENG", "pool")

_CACHE = {}


def _build_program():
    """Build + compile the (single, SPMD) Bass program. Returns nc."""
    from contextlib import ExitStack

    import concourse.bass as bass
    import concourse.tile as tile
    from concourse import bacc, mybir
    from concourse.masks import make_identity

    f32 = mybir.dt.float32
    f32r = mybir.dt.float32r
    bf16 = mybir.dt.bfloat16
    AF = mybir.ActivationFunctionType
    OP = mybir.AluOpType

    # The act-table chooser greedily picks the FIRST set containing each
    # activation function, which alternates between 'exp_and_others' (Exp)
    # and 'natural_log' (Ln) -- a 1.3us table reload per switch.  Both live
    # together in 'natural_log_exp_and_others'; restrict the chooser's view
    # so Exp/Ln only resolve there.  Order and length of the table list are
    # preserved, so the emitted act_func_set_id still indexes the real
    # act_info.json and the loaded table genuinely contains both functions.
    import concourse.bacc as bacc_mod
    from concourse.hw_specs import get_activation_tables as _gat

    def _tables_joint_expln(arch):
        out = {}
        for name, s in _gat(arch).items():
            s2 = set(s)
            if name != "natural_log_exp_and_others":
                s2.discard(AF.Exp)
                s2.discard(AF.Ln)
            out[name] = s2
        return out

    bacc_mod.get_activation_tables = _tables_joint_expln

    nc = bacc.Bacc("TRN2", target_bir_lowering=False)

    xT_d = nc.dram_tensor("xT", [D, S], f32r, kind="ExternalInput").ap()
    wqT_d = nc.dram_tensor("wqT", [D, G * HD], f32r, kind="ExternalInput").ap()
    wkT_d = nc.dram_tensor("wkT", [D, HD], f32r, kind="ExternalInput").ap()
    wvT_d = nc.dram_tensor("wvT", [D, HD], f32r, kind="ExternalInput").ap()
    wpT_d = nc.dram_tensor("wpT", [G * HD, D], bf16, kind="ExternalInput").ap()
    cosT_d = nc.dram_tensor("cosT", [HD, S], bf16, kind="ExternalInput").ap()
    sinT_d = nc.dram_tensor("sinT", [HD, S], bf16, kind="ExternalInput").ap()
    jT_d = nc.dram_tensor("jT", [HD, HD], bf16, kind="ExternalInput").ap()
    lng_d = nc.dram_tensor("lng", [1, G], f32, kind="ExternalInput").ap()
    y_d = nc.dram_tensor("y", [S, D], f32, kind="ExternalOutput").ap()

    xT3 = xT_d.rearrange("(o p) s -> p o s", p=P)

    with tile.TileContext(nc) as tc, ExitStack() as top:
        res = top.enter_context(tc.tile_pool(name="res", bufs=1))
        wpool = top.enter_context(tc.tile_pool(name="w", bufs=1))
        xtp = top.enter_context(tc.tile_pool(name="xt", bufs=3))
        nrm = top.enter_context(tc.tile_pool(name="nrm", bufs=2))
        ptp = top.enter_context(tc.tile_pool(name="pt", bufs=6))
        rbp = top.enter_context(tc.tile_pool(name="rb", bufs=2))
        otp = top.enter_context(tc.tile_pool(name="oT", bufs=2))
        ysp = top.enter_context(tc.tile_pool(name="ys", bufs=2))
        psA = top.enter_context(tc.tile_pool(name="psA", bufs=1, space="PSUM"))
        psB = top.enter_context(tc.tile_pool(name="psB", bufs=1, space="PSUM"))

        # ---- PE warm-up: ramp the p-state clock while DMAs stream in ----
        warm = res.tile([P, 448], bf16)
        nc.vector.memset(warm[:], 0.25)
        ones_bf = res.tile([P, P], bf16)
        nc.vector.memset(ones_bf[:], 1.0)
        for w in range(16):
            wt = psB.tile([P, SL], f32, tag="sc", bufs=SC_BUFS, name=f"warm{w}")
            nc.tensor.matmul(wt[:, 0:448], ones_bf[:], warm[:],
                             start=True, stop=True)

        # ---- input DMAs (issue order matters at startup) ----
        wqT3 = wqT_d.rearrange("(o p) m -> p o m", p=P)
        wk_sb = wpool.tile([P, DK, HD], f32r)
        nc.sync.dma_start(wk_sb[:], wkT_d.rearrange("(o p) m -> p o m", p=P))
        x_lo0 = xtp.tile([P, HK, SL], f32r, tag="xt", name="xlo0")
        nc.sync.dma_start(x_lo0[:, 0:2, :], xT3[:, 0:2, 0:SL])
        wv_sb = wpool.tile([P, DK, HD], f32r)
        nc.sync.dma_start(wv_sb[:], wvT_d.rearrange("(o p) m -> p o m", p=P))
        wq_sb = wpool.tile([P, DK, G * HD], f32r)
        nc.sync.dma_start(wq_sb[:, :, 0:HD], wqT3[:, :, 0:HD])
        cos_sb = res.tile([HD, S], bf16)
        nc.sync.dma_start(cos_sb[:], cosT_d[:])
        sin_sb = res.tile([HD, S], bf16)
        nc.sync.dma_start(sin_sb[:], sinT_d[:])
        nc.sync.dma_start(x_lo0[:, 2:5, :], xT3[:, 2:5, 0:SL])
        nc.sync.dma_start(wq_sb[:, :, HD:2 * HD], wqT3[:, :, HD:2 * HD])
        nc.sync.dma_start(x_lo0[:, 5:8, :], xT3[:, 5:8, 0:SL])
        x_hi0 = xtp.tile([P, HK, SL], f32r, tag="xt", name="xhi0")
        nc.sync.dma_start(x_hi0[:, 0:4, :], xT3[:, HK:HK + 4, 0:SL])
        nc.sync.dma_start(wq_sb[:, :, 2 * HD:3 * HD], wqT3[:, :, 2 * HD:3 * HD])
        nc.sync.dma_start(x_hi0[:, 4:8, :], xT3[:, HK + 4:DK, 0:SL])
        nc.sync.dma_start(wq_sb[:, :, 3 * HD:4 * HD], wqT3[:, :, 3 * HD:4 * HD])
        jT = res.tile([HD, HD], bf16)
        nc.sync.dma_start(jT[:], jT_d[:])
        wp_sb = wpool.tile([P, G, D], bf16)
        nc.sync.dma_start(wp_sb[:], wpT_d.rearrange("(o p) m -> p o m", p=P))
        lng = res.tile([P, G], f32)
        nc.gpsimd.dma_start(lng[:], lng_d.to_broadcast([P, G]))

        # ---- small constants ----
        ident = res.tile([P, P], bf16)
        make_identity(nc, ident[:])
        eps_t = res.tile([P, 1], f32)
        nc.vector.memset(eps_t[:], EPS)
        # additive causal mask for diagonal 128-key blocks: column c (local,
        # after the lo_c shift) of partition r is live iff c >= r
        mask = res.tile([P, SL], bf16)
        nc.vector.memset(mask[:], 0.0)
        nc.gpsimd.affine_select(out=mask[:], in_=mask[:], compare_op=OP.is_ge,
                                fill=-1e30, base=0, pattern=[[1, SL]],
                                channel_multiplier=-1)

        # ---- resident K^T / Q^T / V ----
        kT = res.tile([P, S], bf16)
        qT = [res.tile([P, S], bf16, name=f"qT{h}") for h in range(G)]
        v_sb = res.tile([P, S // P, HD], bf16)

        def norm_rope(src_ps, dst, js, lng_ap):
            """RMS-normalize (+gain) and RoPE a [128, SL] head block.

            fb = gain * (mean(src^2) + eps)^-1/2 via exp(-0.5*ln(.) + ln(gain))
            rot = src*cos + (J@src)*sin, dst = rot*fb  (rope commutes with the
            per-token scaling, so normalize last).
            """
            src = nrm.tile([P, SL], bf16, tag="src")
            nc.scalar.copy(src[:], src_ps[:])
            sq = nrm.tile([P, SL], bf16, tag="sq")
            nc.vector.tensor_mul(sq[:], src[:], src[:])
            ssq = psA.tile([P, SL], f32, tag="nrm", bufs=NRM_BUFS)
            nc.tensor.matmul(ssq[:], ones_bf[:], sq[:], start=True, stop=True)
            qj = psA.tile([P, SL], f32, tag="nrm", bufs=NRM_BUFS)
            nc.tensor.matmul(qj[:], jT[:], src[:], start=True, stop=True)
            u = nrm.tile([P, SL], f32, tag="u")
            nc.scalar.activation(u[:], ssq[:], AF.Ln, bias=eps_t[:],
                                 scale=1.0 / HD)
            fb = nrm.tile([P, SL], bf16, tag="fb")
            nc.scalar.activation(fb[:], u[:], AF.Exp, scale=-0.5,
                                 bias=lng_ap if lng_ap is not None else 0.0)
            c = cos_sb[:, js * SL:(js + 1) * SL]
            s = sin_sb[:, js * SL:(js + 1) * SL]
            t1 = nrm.tile([P, SL], bf16, tag="t1")
            nc.vector.tensor_mul(t1[:], src[:], c)
            t2 = nrm.tile([P, SL], bf16, tag="t2")
            nc.vector.tensor_mul(t2[:], qj[:], s)
            rot = nrm.tile([P, SL], bf16, tag="rot")
            nc.vector.tensor_add(rot[:], t1[:], t2[:])
            nc.vector.tensor_mul(dst, rot[:], fb[:])

        for js in range(NSL):
            # ---- prefetch x for js+1 ----
            if js == 0:
                x_lo, x_hi = x_lo0, x_hi0
            if js + 1 < NSL:
                ssl = slice((js + 1) * SL, (js + 2) * SL)
                x_lo_n = xtp.tile([P, HK, SL], f32r, tag="xt",
                                  name=f"xlo{js + 1}")
                nc.sync.dma_start(x_lo_n[:, 0:4, :], xT3[:, 0:4, ssl])
                nc.sync.dma_start(x_lo_n[:, 4:8, :], xT3[:, 4:HK, ssl])
                x_hi_n = xtp.tile([P, HK, SL], f32r, tag="xt",
                                  name=f"xhi{js + 1}")
                nc.sync.dma_start(x_hi_n[:, 0:4, :], xT3[:, HK:HK + 4, ssl])
                nc.sync.dma_start(x_hi_n[:, 4:8, :], xT3[:, HK + 4:DK, ssl])

            def xts(dk):
                return (x_lo if dk < HK else x_hi)[:, dk % HK, :]

            # ---- A: QKV projections for slice js ----
            dsl = slice(js * SL, (js + 1) * SL)
            k_ps = psA.tile([P, SL], f32, tag="qkv", bufs=QKV_BUFS)
            for dk in range(DK):
                nc.tensor.matmul(k_ps[:], wk_sb[:, dk, :], xts(dk),
                                 start=(dk == 0), stop=(dk == DK - 1))
            norm_rope(k_ps, kT[:, dsl], js, None)

            v_ps = psA.tile([P, SL], f32, tag="qkv", bufs=QKV_BUFS)
            for dk in range(DK):
                nc.tensor.matmul(v_ps[:], wv_sb[:, dk, :], xts(dk),
                                 start=(dk == 0), stop=(dk == DK - 1))
            vt = nrm.tile([P, SL], bf16, tag="vt")
            nc.scalar.copy(vt[:], v_ps[:])
            vtr = psA.tile([P, 4, P], bf16, tag="nrm", bufs=NRM_BUFS)
            for t in range(4):
                nc.tensor.transpose(vtr[:, t, :], vt[:, t * P:(t + 1) * P],
                                    ident[:])
            nc.scalar.copy(v_sb[:, js * 4:(js + 1) * 4, :], vtr[:])

            for h in range(G):
                if js == 0 and h < 3:
                    q_ps = psB.tile([P, SL], f32, tag="sc", bufs=SC_BUFS,
                                    name=f"q0_{h}_ps")
                else:
                    q_ps = psA.tile([P, SL], f32, tag="qkv", bufs=QKV_BUFS)
                for dk in range(DK):
                    nc.tensor.matmul(q_ps[:],
                                     wq_sb[:, dk, h * HD:(h + 1) * HD],
                                     xts(dk), start=(dk == 0),
                                     stop=(dk == DK - 1))
                norm_rope(q_ps, qT[h][:, dsl], js, lng[:, h:h + 1])

            if js + 1 < NSL:
                x_lo, x_hi = x_lo_n, x_hi_n

            # ---- B: attention for query slice jq = js ----
            jq = js
            ilast = 4 * jq + 3
            for h in range(G):
                if h % 2 == 0:
                    rs_ps = psB.tile([P, SL], f32, tag="rs", bufs=1)
                    o_ps = psB.tile([P, SL], f32, tag="o", bufs=1)
                else:
                    rs_ps = psA.tile([P, SL], f32, tag="qkv", bufs=QKV_BUFS,
                                     name=f"rs_{js}_{h}")
                    o_ps = psA.tile([P, SL], f32, tag="qkv", bufs=QKV_BUFS,
                                    name=f"o_{js}_{h}")
                for i in range(ilast + 1):
                    delta = i - 4 * jq
                    lo_c = P * delta if delta >= 0 else 0
                    sp = slice(lo_c, SL)
                    qsp = slice(jq * SL + lo_c, (jq + 1) * SL)
                    sc = psB.tile([P, SL], f32, tag="sc", bufs=SC_BUFS)
                    if delta >= 0:
                        meng = nc.gpsimd if MASK_ENG == "pool" else (
                            nc.vector if MASK_ENG == "dve" else nc.scalar)
                        meng.tensor_copy(sc[:, sp], mask[:, 0:SL - lo_c])
                    nc.tensor.matmul(sc[:, sp], kT[:, i * P:(i + 1) * P],
                                     qT[h][:, qsp], start=(delta < 0),
                                     stop=True, skip_group_check=True)
                    pt = ptp.tile([P, SL], bf16, tag="pt")
                    nc.scalar.activation(pt[:, sp], sc[:, sp], AF.Exp)
                    stop = i == ilast
                    nc.tensor.matmul(rs_ps[:, sp], ones_bf[:], pt[:, sp],
                                     start=(i == 0), stop=stop)
                    nc.tensor.matmul(o_ps[:, sp], v_sb[:, i, :], pt[:, sp],
                                     start=(i == 0), stop=stop)
                rb = rbp.tile([P, SL], f32, tag="rb")
                nc.vector.reciprocal(rb[:], rs_ps[:])
                if h == 0:
                    oT = otp.tile([P, G, SL], bf16, tag="oT")
                nc.vector.tensor_mul(oT[:, h, :], o_ps[:], rb[:])

            # ---- C: output projection for token slice js ----
            for st in range(SL // P):
                st_glob = js * 4 + st
                tsl = slice(st * P, (st + 1) * P)
                for os_ in range(4):
                    y_ps = psB.tile([P, SL], f32, tag="sc", bufs=SC_BUFS,
                                    name=f"y_{js}_{st}_{os_}")
                    for h in range(G):
                        nc.tensor.matmul(
                            y_ps[:], oT[:, h, tsl],
                            wp_sb[:, h, os_ * SL:(os_ + 1) * SL],
                            start=(h == 0), stop=(h == G - 1))
                    y_sb = ysp.tile([P, SL], f32, tag="ys", bufs=4)
                    if os_ % 2 == 0:
                        nc.gpsimd.tensor_copy(y_sb[:], y_ps[:])
                    else:
                        nc.vector.tensor_copy(y_sb[:], y_ps[:])
                    nc.sync.dma_start(
                        y_d[st_glob * P:(st_glob + 1) * P,
                            os_ * SL:(os_ + 1) * SL], y_sb[:])

    nc.compile()
    return nc


def _rope_tables():
    """cos/sin tables in [HD, S] layout (half-tables stacked twice), plus J^T."""
    inv_freq = 1.0 / (ROPE_BASE ** (np.arange(0, HD, 2, dtype=np.float32) / HD))
    freqs = np.outer(np.arange(S, dtype=np.float32), inv_freq)  # [S, half]
    c = np.cos(freqs).T.astype(np.float32)  # [half, S]
    s = np.sin(freqs).T.astype(np.float32)
    cosf = np.concatenate([c, c], axis=0).copy()  # [HD, S]
    sinf = np.concatenate([s, s], axis=0).copy()
    half = HD // 2
    jT = np.zeros((HD, HD), np.float32)
    jT[np.arange(half) + half, np.arange(half)] = 1.0   # (Jq)[j] = q[j+64], j<64
    jT[np.arange(half), np.arange(half) + half] = -1.0  # (Jq)[j+64] = -q[j]
    return cosf, sinf, jT


def make_in_maps(x, Wq, Wk, Wv, Wproj, q_gain):
    """Host-side shard prep: per-core input dicts."""
    import ml_dtypes

    bf = ml_dtypes.bfloat16
    cosT, sinT, jT = _rope_tables()
    cosT, sinT, jT = cosT.astype(bf), sinT.astype(bf), jT.astype(bf)
    xT = np.ascontiguousarray(np.transpose(np.asarray(x, np.float32), (0, 2, 1)))
    Wq = np.asarray(Wq, np.float32)
    Wk = np.asarray(Wk, np.float32)
    Wv = np.asarray(Wv, np.float32)
    WpT = np.ascontiguousarray(np.asarray(Wproj, np.float32).T)  # [in, out]
    q_gain = np.asarray(q_gain, np.float32)

    in_maps = []
    for c in range(NCORES):
        b, g = divmod(c, KVH)
        sl_q = slice(g * G * HD, (g + 1) * G * HD)
        sl_kv = slice(g * HD, (g + 1) * HD)
        in_maps.append({
            "xT": xT[b],
            "wqT": np.ascontiguousarray(Wq[sl_q, :].T),
            "wkT": np.ascontiguousarray(Wk[sl_kv, :].T),
            "wvT": np.ascontiguousarray(Wv[sl_kv, :].T),
            "wpT": np.ascontiguousarray(WpT[sl_q, :]).astype(bf),
            "cosT": cosT,
            "sinT": sinT,
            "jT": jT,
            "lng": np.log(q_gain[g * G:(g + 1) * G] / np.sqrt(HD))
            .reshape(1, G).astype(np.float32),
        })
    return in_maps


def kernel(x, Wq, Wk, Wv, Wproj, q_gain):
    from concourse.bass_utils import run_bass_kernel_spmd

    if "nc" not in _CACHE:
        _CACHE["nc"] = _build_program()
    nc = _CACHE["nc"]

    in_maps = make_in_maps(x, Wq, Wk, Wv, Wproj, q_gain)
    res = run_bass_kernel_spmd(nc, in_maps, core_ids=list(range(NCORES)))
    _CACHE["last_results"] = res

    y = np.zeros((B, S, D), dtype=np.float32)
    for c in range(NCORES):
        y[c // KVH] += res.results[c]["y"]
    return y


# revision 8
# speedup vs baseline: 1.0441x; 1.0441x over previous
"""Causal self-attention (GQA + QK-RMSNorm + RoPE + q_gain) on 8 Trainium2 cores.

Sharding: 8 cores = 2 (batch) x 4 (KV head group).  Core c handles batch
c//4 and KV head g=c%4, i.e. Q heads 4g..4g+3.  Each core computes its
heads' attention and a partial output projection (its 512 columns of the
attention output against the matching 512 rows of Wproj^T); the host sums
the 4 partials per batch.

Single streamed loop over 512-token slices: QKV projection (+ QK-RMSNorm +
RoPE) for slice js, then causal attention for query slice js (keys 0..js),
then the output projection for those tokens.  The Tile scheduler overlaps
the phases across slices.

Datapath: fp32r for the QKV projections (x and W stay fp32), bf16 for
q/k/v, attention probabilities, attention output and Wproj (all matmuls
stay at 1 PE cycle per output column; bf16 doubles DVE throughput and
halves SBUF).  The causal mask is preloaded into PSUM (additive -1e30)
so masking costs nothing on the exp->PV critical chain.  RMSNorm gain and
1/sqrt(hd) are folded into exp(-0.5*ln(z)) on the scalar engine, which
also keeps every activation in one table set (no table reloads).

All shapes hardcoded for B=2, S=2048, D=2048, H=16, KVH=4, HD=128.
"""

import os

import numpy as np

B, S, D = 2, 2048, 2048
H, KVH = 16, 4
HD = 128  # head dim
G = H // KVH  # q heads per kv group = 4
NCORES = 8
ROPE_BASE = 10000.0
EPS = 1e-6

P = 128          # partitions
SL = 512         # token slice
NSL = S // SL    # 4
DK = D // P      # 16 contraction subtiles
HK = DK // 2     # x arrives in half-slices of 8 subtiles

# psum pool buffer knobs (env-overridable for perf sweeps; defaults are the
# shipped configuration and must sum to <= 8 banks with rs+o+vtr)
QKV_BUFS = int(os.environ.get("K_QKV", "2"))
NRM_BUFS = int(os.environ.get("K_NRM", "2"))
SC_BUFS = int(os.environ.get("K_SC", "2"))
MASK_ENG = os.environ.get("K_MASKENG", "pool")

_CACHE = {}


def _build_program():
    """Build + compile the (single, SPMD) Bass program. Returns nc."""
    from contextlib import ExitStack

    import concourse.bass as bass
    import concourse.tile as tile
    from concourse import bacc, mybir
    from concourse.masks import make_identity

    f32 = mybir.dt.float32
    f32r = mybir.dt.float32r
    bf16 = mybir.dt.bfloat16
    AF = mybir.ActivationFunctionType
    OP = mybir.AluOpType

    # The act-table chooser greedily picks the FIRST set containing each
    # activation function, which alternates between 'exp_and_others' (Exp)
    # and 'natural_log' (Ln) -- a 1.3us table reload per switch.  Both live
    # together in 'natural_log_exp_and_others'; restrict the chooser's view
    # so Exp/Ln only resolve there.  Order and length of the table list are
    # preserved, so the emitted act_func_set_id still indexes the real
    # act_info.json and the loaded table genuinely contains both functions.
    import concourse.bacc as bacc_mod
    from concourse.hw_specs import get_activation_tables as _gat

    def _tables_joint_expln(arch):
        out = {}
        for name, s in _gat(arch).items():
            s2 = set(s)
            if name != "natural_log_exp_and_others":
                s2.discard(AF.Exp)
                s2.discard(AF.Ln)
            out[name] = s2
        return out

    bacc_mod.get_activation_tables = _tables_joint_expln

    nc = bacc.Bacc("TRN2", target_bir_lowering=False)

    xT_d = nc.dram_tensor("xT", [D, S], f32r, kind="ExternalInput").ap()
    wqT_d = nc.dram_tensor("wqT", [D, G * HD], f32r, kind="ExternalInput").ap()
    wkT_d = nc.dram_tensor("wkT", [D, HD], f32r, kind="ExternalInput").ap()
    wvT_d = nc.dram_tensor("wvT", [D, HD], f32r, kind="ExternalInput").ap()
    wpT_d = nc.dram_tensor("wpT", [G * HD, D], bf16, kind="ExternalInput").ap()
    cosT_d = nc.dram_tensor("cosT", [HD, S], bf16, kind="ExternalInput").ap()
    sinT_d = nc.dram_tensor("sinT", [HD, S], bf16, kind="ExternalInput").ap()
    jT_d = nc.dram_tensor("jT", [HD, HD], bf16, kind="ExternalInput").ap()
    lng_d = nc.dram_tensor("lng", [1, G], f32, kind="ExternalInput").ap()
    y_d = nc.dram_tensor("y", [S, D], f32, kind="ExternalOutput").ap()

    xT3 = xT_d.rearrange("(o p) s -> p o s", p=P)

    with tile.TileContext(nc) as tc, ExitStack() as top:
        res = top.enter_context(tc.tile_pool(name="res", bufs=1))
        wpool = top.enter_context(tc.tile_pool(name="w", bufs=1))
        xtp = top.enter_context(tc.tile_pool(name="xt", bufs=3))
        nrm = top.enter_context(tc.tile_pool(name="nrm", bufs=2))
        ptp = top.enter_context(tc.tile_pool(name="pt", bufs=6))
        rbp = top.enter_context(tc.tile_pool(name="rb", bufs=2))
        otp = top.enter_context(tc.tile_pool(name="oT", bufs=2))
        ysp = top.enter_context(tc.tile_pool(name="ys", bufs=2))
        psA = top.enter_context(tc.tile_pool(name="psA", bufs=1, space="PSUM"))
        psB = top.enter_context(tc.tile_pool(name="psB", bufs=1, space="PSUM"))

        # ---- PE warm-up: ramp the p-state clock while DMAs stream in ----
        warm = res.tile([P, 448], bf16)
        nc.vector.memset(warm[:], 0.25)
        ones_bf = res.tile([P, P], bf16)
        nc.vector.memset(ones_bf[:], 1.0)
        for w in range(16):
            wt = psB.tile([P, SL], f32, tag="sc", bufs=SC_BUFS, name=f"warm{w}")
            nc.tensor.matmul(wt[:, 0:448], ones_bf[:], warm[:],
                             start=True, stop=True)

        # ---- input DMAs (issue order matters at startup) ----
        wqT3 = wqT_d.rearrange("(o p) m -> p o m", p=P)
        wk_sb = wpool.tile([P, DK, HD], f32r)
        nc.sync.dma_start(wk_sb[:], wkT_d.rearrange("(o p) m -> p o m", p=P))
        x_lo0 = xtp.tile([P, HK, SL], f32r, tag="xt", name="xlo0")
        nc.sync.dma_start(x_lo0[:, 0:2, :], xT3[:, 0:2, 0:SL])
        wv_sb = wpool.tile([P, DK, HD], f32r)
        nc.sync.dma_start(wv_sb[:], wvT_d.rearrange("(o p) m -> p o m", p=P))
        wq_sb = wpool.tile([P, DK, G * HD], f32r)
        nc.sync.dma_start(wq_sb[:, :, 0:HD], wqT3[:, :, 0:HD])
        cos_sb = res.tile([HD, S], bf16)
        nc.sync.dma_start(cos_sb[:], cosT_d[:])
        sin_sb = res.tile([HD, S], bf16)
        nc.sync.dma_start(sin_sb[:], sinT_d[:])
        nc.sync.dma_start(x_lo0[:, 2:5, :], xT3[:, 2:5, 0:SL])
        nc.sync.dma_start(wq_sb[:, :, HD:2 * HD], wqT3[:, :, HD:2 * HD])
        nc.sync.dma_start(x_lo0[:, 5:8, :], xT3[:, 5:8, 0:SL])
        x_hi0 = xtp.tile([P, HK, SL], f32r, tag="xt", name="xhi0")
        nc.sync.dma_start(x_hi0[:, 0:4, :], xT3[:, HK:HK + 4, 0:SL])
        nc.sync.dma_start(wq_sb[:, :, 2 * HD:3 * HD], wqT3[:, :, 2 * HD:3 * HD])
        nc.sync.dma_start(x_hi0[:, 4:8, :], xT3[:, HK + 4:DK, 0:SL])
        nc.sync.dma_start(wq_sb[:, :, 3 * HD:4 * HD], wqT3[:, :, 3 * HD:4 * HD])
        jT = res.tile([HD, HD], bf16)
        nc.sync.dma_start(jT[:], jT_d[:])
        wp_sb = wpool.tile([P, G, D], bf16)
        nc.sync.dma_start(wp_sb[:], wpT_d.rearrange("(o p) m -> p o m", p=P))
        lng = res.tile([P, G], f32)
        nc.gpsimd.dma_start(lng[:], lng_d.to_broadcast([P, G]))

        # ---- small constants ----
        ident = res.tile([P, P], bf16)
        make_identity(nc, ident[:])
        eps_t = res.tile([P, 1], f32)
        nc.vector.memset(eps_t[:], EPS)
        # additive causal mask for diagonal 128-key blocks: column c (local,
        # after the lo_c shift) of partition r is live iff c >= r
        mask = res.tile([P, SL], bf16)
        nc.vector.memset(mask[:], 0.0)
        nc.gpsimd.affine_select(out=mask[:], in_=mask[:], compare_op=OP.is_ge,
                                fill=-1e30, base=0, pattern=[[1, SL]],
                                channel_multiplier=-1)

        # ---- resident K^T / Q^T / V ----
        kT = res.tile([P, S], bf16)
        qT = [res.tile([P, S], bf16, name=f"qT{h}") for h in range(G)]
        v_sb = res.tile([P, S // P, HD], bf16)

        def norm_rope(src_ps, dst, js, lng_ap):
            """RMS-normalize (+gain) and RoPE a [128, SL] head block.

            fb = gain * (mean(src^2) + eps)^-1/2 via exp(-0.5*ln(.) + ln(gain))
            rot = src*cos + (J@src)*sin, dst = rot*fb  (rope commutes with the
            per-token scaling, so normalize last).
            """
            src = nrm.tile([P, SL], bf16, tag="src")
            nc.scalar.copy(src[:], src_ps[:])
            sq = nrm.tile([P, SL], bf16, tag="sq")
            nc.vector.tensor_mul(sq[:], src[:], src[:])
            ssq = psA.tile([P, SL], f32, tag="nrm", bufs=NRM_BUFS)
            nc.tensor.matmul(ssq[:], ones_bf[:], sq[:], start=True, stop=True)
            qj = psA.tile([P, SL], f32, tag="nrm", bufs=NRM_BUFS)
            nc.tensor.matmul(qj[:], jT[:], src[:], start=True, stop=True)
            u = nrm.tile([P, SL], f32, tag="u")
            nc.scalar.activation(u[:], ssq[:], AF.Ln, bias=eps_t[:],
                                 scale=1.0 / HD)
            fb = nrm.tile([P, SL], bf16, tag="fb")
            nc.scalar.activation(fb[:], u[:], AF.Exp, scale=-0.5,
                                 bias=lng_ap if lng_ap is not None else 0.0)
            c = cos_sb[:, js * SL:(js + 1) * SL]
            s = sin_sb[:, js * SL:(js + 1) * SL]
            t1 = nrm.tile([P, SL], bf16, tag="t1")
            nc.vector.tensor_mul(t1[:], src[:], c)
            t2 = nrm.tile([P, SL], bf16, tag="t2")
            nc.vector.tensor_mul(t2[:], qj[:], s)
            rot = nrm.tile([P, SL], bf16, tag="rot")
            nc.vector.tensor_add(rot[:], t1[:], t2[:])
            nc.vector.tensor_mul(dst, rot[:], fb[:])

        for js in range(NSL):
            # ---- prefetch x for js+1 ----
            if js == 0:
                x_lo, x_hi = x_lo0, x_hi0
            if js + 1 < NSL:
                ssl = slice((js + 1) * SL, (js + 2) * SL)
                x_lo_n = xtp.tile([P, HK, SL], f32r, tag="xt",
                                  name=f"xlo{js + 1}")
                nc.sync.dma_start(x_lo_n[:, 0:4, :], xT3[:, 0:4, ssl])
                nc.sync.dma_start(x_lo_n[:, 4:8, :], xT3[:, 4:HK, ssl])
                x_hi_n = xtp.tile([P, HK, SL], f32r, tag="xt",
                                  name=f"xhi{js + 1}")
                nc.sync.dma_start(x_hi_n[:, 0:4, :], xT3[:, HK:HK + 4, ssl])
                nc.sync.dma_start(x_hi_n[:, 4:8, :], xT3[:, HK + 4:DK, ssl])

            def xts(dk):
                return (x_lo if dk < HK else x_hi)[:, dk % HK, :]

            # ---- A: QKV projections for slice js ----
            dsl = slice(js * SL, (js + 1) * SL)
            k_ps = psA.tile([P, SL], f32, tag="qkv", bufs=QKV_BUFS)
            for dk in range(DK):
                nc.tensor.matmul(k_ps[:], wk_sb[:, dk, :], xts(dk),
                                 start=(dk == 0), stop=(dk == DK - 1))
            norm_rope(k_ps, kT[:, dsl], js, None)

            v_ps = psA.tile([P, SL], f32, tag="qkv", bufs=QKV_BUFS)
            for dk in range(DK):
                nc.tensor.matmul(v_ps[:], wv_sb[:, dk, :], xts(dk),
                                 start=(dk == 0), stop=(dk == DK - 1))
            vt = nrm.tile([P, SL], bf16, tag="vt")
            nc.scalar.copy(vt[:], v_ps[:])
            vtr = psA.tile([P, 4, P], bf16, tag="nrm", bufs=NRM_BUFS)
            for t in range(4):
                nc.tensor.transpose(vtr[:, t, :], vt[:, t * P:(t + 1) * P],
                                    ident[:])
            nc.scalar.copy(v_sb[:, js * 4:(js + 1) * 4, :], vtr[:])

            for h in range(G):
                if js == 0 and h < 3:
                    q_ps = psB.tile([P, SL], f32, tag="sc", bufs=SC_BUFS,
                                    name=f"q0_{h}_ps")
                else:
                    q_ps = psA.tile([P, SL], f32, tag="qkv", bufs=QKV_BUFS)
                for dk in range(DK):
                    nc.tensor.matmul(q_ps[:],
                                     wq_sb[:, dk, h * HD:(h + 1) * HD],
                                     xts(dk), start=(dk == 0),
                                     stop=(dk == DK - 1))
                norm_rope(q_ps, qT[h][:, dsl], js, lng[:, h:h + 1])

            if js + 1 < NSL:
                x_lo, x_hi = x_lo_n, x_hi_n

            # ---- B: attention for query slice jq = js ----
            jq = js
            ilast = 4 * jq + 3
            for h in range(G):
                rs_ps = psB.tile([P, SL], f32, tag="rs", bufs=1)
                o_ps = psB.tile([P, SL], f32, tag="o", bufs=1)
                for i in range(ilast + 1):
                    delta = i - 4 * jq
                    lo_c = P * delta if delta >= 0 else 0
                    sp = slice(lo_c, SL)
                    qsp = slice(jq * SL + lo_c, (jq + 1) * SL)
                    sc = psB.tile([P, SL], f32, tag="sc", bufs=SC_BUFS)
                    if delta >= 0:
                        meng = nc.gpsimd if MASK_ENG == "pool" else (
                            nc.vector if MASK_ENG == "dve" else nc.scalar)
                        meng.tensor_copy(sc[:, sp], mask[:, 0:SL - lo_c])
                    nc.tensor.matmul(sc[:, sp], kT[:, i * P:(i + 1) * P],
                                     qT[h][:, qsp], start=(delta < 0),
                                     stop=True, skip_group_check=True)
                    pt = ptp.tile([P, SL], bf16, tag="pt")
                    nc.scalar.activation(pt[:, sp], sc[:, sp], AF.Exp)
                    stop = i == ilast
                    nc.tensor.matmul(rs_ps[:, sp], ones_bf[:], pt[:, sp],
                                     start=(i == 0), stop=stop)
                    nc.tensor.matmul(o_ps[:, sp], v_sb[:, i, :], pt[:, sp],
                                     start=(i == 0), stop=stop)
                rb = rbp.tile([P, SL], f32, tag="rb")
                nc.vector.reciprocal(rb[:], rs_ps[:])
                if h == 0:
                    oT = otp.tile([P, G, SL], bf16, tag="oT")
                nc.vector.tensor_mul(oT[:, h, :], o_ps[:], rb[:])

            # ---- C: output projection for token slice js ----
            for st in range(SL // P):
                st_glob = js * 4 + st
                tsl = slice(st * P, (st + 1) * P)
                for os_ in range(4):
                    y_ps = psB.tile([P, SL], f32, tag="sc", bufs=SC_BUFS,
                                    name=f"y_{js}_{st}_{os_}")
                    for h in range(G):
                        nc.tensor.matmul(
                            y_ps[:], oT[:, h, tsl],
                            wp_sb[:, h, os_ * SL:(os_ + 1) * SL],
                            start=(h == 0), stop=(h == G - 1))
                    y_sb = ysp.tile([P, SL], f32, tag="ys", bufs=4)
                    if os_ % 2 == 0:
                        nc.gpsimd.tensor_copy(y_sb[:], y_ps[:])
                    else:
                        nc.vector.tensor_copy(y_sb[:], y_ps[:])
                    nc.sync.dma_start(
                        y_d[st_glob * P:(st_glob + 1) * P,
                            os_ * SL:(os_ + 1) * SL], y_sb[:])

    nc.compile()
    return nc


def _rope_tables():
    """cos/sin tables in [HD, S] layout (half-tables stacked twice), plus J^T."""
    inv_freq = 1.0 / (ROPE_BASE ** (np.arange(0, HD, 2, dtype=np.float32) / HD))
    freqs = np.outer(np.arange(S, dtype=np.float32), inv_freq)  # [S, half]
    c = np.cos(freqs).T.astype(np.float32)  # [half, S]
    s = np.sin(freqs).T.astype(np.float32)
    cosf = np.concatenate([c, c], axis=0).copy()  # [HD, S]
    sinf = np.concatenate([s, s], axis=0).copy()
    half = HD // 2
    jT = np.zeros((HD, HD), np.float32)
    jT[np.arange(half) + half, np.arange(half)] = 1.0   # (Jq)[j] = q[j+64], j<64
    jT[np.arange(half), np.arange(half) + half] = -1.0  # (Jq)[j+64] = -q[j]
    return cosf, sinf, jT


def make_in_maps(x, Wq, Wk, Wv, Wproj, q_gain):
    """Host-side shard prep: per-core input dicts."""
    import ml_dtypes

    bf = ml_dtypes.bfloat16
    cosT, sinT, jT = _rope_tables()
    cosT, sinT, jT = cosT.astype(bf), sinT.astype(bf), jT.astype(bf)
    xT = np.ascontiguousarray(np.transpose(np.asarray(x, np.float32), (0, 2, 1)))
    Wq = np.asarray(Wq, np.float32)
    Wk = np.asarray(Wk, np.float32)
    Wv = np.asarray(Wv, np.float32)
    WpT = np.ascontiguousarray(np.asarray(Wproj, np.float32).T)  # [in, out]
    q_gain = np.asarray(q_gain, np.float32)

    in_maps = []
    for c in range(NCORES):
        b, g = divmod(c, KVH)
        sl_q = slice(g * G * HD, (g + 1) * G * HD)
        sl_kv = slice(g * HD, (g + 1) * HD)
        in_maps.append({
            "xT": xT[b],
            "wqT": np.ascontiguousarray(Wq[sl_q, :].T),
            "wkT": np.ascontiguousarray(Wk[sl_kv, :].T),
            "wvT": np.ascontiguousarray(Wv[sl_kv, :].T),
            "wpT": np.ascontiguousarray(WpT[sl_q, :]).astype(bf),
            "cosT": cosT,
            "sinT": sinT,
            "jT": jT,
            "lng": np.log(q_gain[g * G:(g + 1) * G] / np.sqrt(HD))
            .reshape(1, G).astype(np.float32),
        })
    return in_maps


def kernel(x, Wq, Wk, Wv, Wproj, q_gain):
    from concourse.bass_utils import run_bass_kernel_spmd

    if "nc" not in _CACHE:
        _CACHE["nc"] = _build_program()
    nc = _CACHE["nc"]

    in_maps = make_in_maps(x, Wq, Wk, Wv, Wproj, q_gain)
    res = run_bass_kernel_spmd(nc, in_maps, core_ids=list(range(NCORES)))
    _CACHE["last_results"] = res

    y = np.zeros((B, S, D), dtype=np.float32)
    for c in range(NCORES):
        y[c // KVH] += res.results[c]["y"]
    return y
